# revision 89
# baseline (speedup 1.0000x reference)
"""Multi-head attention (B=2, N=2048, C=768, H=12) on 8 Trainium2 NeuronCores.

Sharding: core c handles batch b=c//4 and head-group g=c%4 (3 heads, 192 dims).
Host side compacts rows where mask==0 out of x (a fully-masked query reduces
to the uniform-attention mean-value row, computed on the host in fp32),
packs every weight slice into its exact SBUF layout (one fast DMA each),
and casts matmul operands to bf16.

Device per core:
  q_T/k_T = W.T @ xcT per 512-col chunk             (q/k head-2 fused in one
                                                     stationary, halving the
                                                     64-row matmul waste)
  scores_T[k, q] = k_T.T-slice @ q_T                (keys on partitions)
  attn_T = exp(0.125*scores + bias[key])            (bias=-1000 marks pad keys)
  outT[65, q] = sum_kt v_aug[kt].T @ attn_T[kt]     (row 64 = softmax sums;
                                                     v projection rides head
                                                     0's exp bubbles, solo/
                                                     q-tail chunks head 1's)
  out_norm = outT[:64] * bcast(1/sums)
  proj[q, 768] = pair + solo65 matmuls              (row 64 of the solo wo
                                                     slice is bo/4, hit by a
                                                     constant ones row)
  bf16 ReduceScatter(add) over the 4 cores of the batch, then a row-wise
  rs_out -> out copy (RS buffers carry 16 pad columns so this copy keeps a
  2D descriptor pattern).
"""

import functools
import numpy as np
import ml_dtypes

import concourse.tile as tile
import concourse.mybir as mybir
from concourse import bacc
from concourse.bass_utils import run_bass_kernel_spmd

B, N, C = 2, 2048, 768
H, D = 12, 64
NCORES, NGROUPS, HPG = 8, 4, 3     # 4 head-groups of 3 heads; 2 batches
HD = HPG * D                       # 192 head dims per core
SCALE = float(D) ** -0.5           # 0.125
SPAN0 = 8                          # span0 width in 128-col units
CT = C // 128                      # 6 contraction tiles of 128
BF16 = mybir.dt.bfloat16
F32 = mybir.dt.float32
NPBF16 = ml_dtypes.bfloat16
RSDT = BF16                        # collective payload dtype

LAST_HW_NS = None
LAST_RESULT = None


def _qchunks(kp, cnt, width):
    """[(start, size)] covering the real (un-padded) query rows [0, cnt)
    in blocks of `width`."""
    out = []
    s = 0
    while s < cnt:
        out.append((s, min(width, cnt - s)))
        s += width
    return out


@functools.lru_cache(maxsize=4)
def _build(kp, cnt_max, reps=1, with_rs=True):
    """Build + compile the SPMD program for padded kept-count `kp`."""
    kt_n = kp // 128
    nc = bacc.Bacc("TRN2", target_bir_lowering=False, debug=False,
                   num_devices=NCORES)

    # all weight inputs are host-packed into their exact SBUF layouts so each
    # loads with one DMA whose contiguous runs are >=512B (no 2x DMA penalty)
    xcT = nc.dram_tensor("xcT", [C, kp], BF16, kind="ExternalInput").ap()
    kcb = nc.dram_tensor("kcb", [128, kt_n], F32, kind="ExternalInput").ap()
    wqP = nc.dram_tensor("wqP", [128, CT * 128], BF16, kind="ExternalInput").ap()
    wkP = nc.dram_tensor("wkP", [128, CT * 128], BF16, kind="ExternalInput").ap()
    wqkS = nc.dram_tensor("wqkS", [128, CT * 128], BF16,
                          kind="ExternalInput").ap()
    wvP = nc.dram_tensor("wvP", [128, CT * HD], BF16, kind="ExternalInput").ap()
    woP = nc.dram_tensor("woP", [128, 2 * C], BF16, kind="ExternalInput").ap()

    out_rows = kp // 4
    out_ext = nc.dram_tensor("out", [out_rows, C], RSDT,
                             kind="ExternalOutput").ap()

    with tile.TileContext(nc) as tc:
        for _ in range(reps):
            _emit(tc, nc, kp, cnt_max, kt_n,
                  xcT, kcb, wqP, wkP, wqkS, wvP, woP, out_ext,
                  with_rs=with_rs)
    nc.compile()
    return nc


def _emit(tc, nc, kp, cnt, kt_n,
          xcT, kcb, wqP, wkP, wqkS, wvP, woP, out_ext, with_rs=True):
    # Span structure: a big span0 keeps the exp/PE pipeline wide; the small
    # span1 overlaps span0's projection + output DMA.
    if cnt > 1024:
        span_w = max(512, min(1024, SPAN0 * 128))
    else:
        span_w = min(cnt, 1024)
    spans = _qchunks(kp, cnt, span_w)
    span0_end = spans[0][0] + spans[0][1]

    with tc.tile_pool(name="const", bufs=1) as consts, \
         tc.tile_pool(name="dram", bufs=1, space="DRAM") as dram, \
         tc.tile_pool(name="s_ps", bufs=2, space="PSUM") as aps, \
         tc.tile_pool(name="o_ps", bufs=1, space="PSUM") as ops, \
         tc.tile_pool(name="p512", bufs=2, space="PSUM") as p512, \
         tc.tile_pool(name="att_sb", bufs=3) as asb, \
         tc.tile_pool(name="n_sb", bufs=27) as nsb, \
         tc.tile_pool(name="on_sb", bufs=2) as onsb, \
         tc.tile_pool(name="pj_sb", bufs=4) as jsb:

        # ---- static loads (issue order == DMA service order) -------------
        # Weights arrive host-packed in SBUF layout: one DMA each, >=512B
        # contiguous runs (no small-element DMA penalty).
        xcT_t = xcT.rearrange("(t p) n -> t p n", p=128)
        wk_sb = consts.tile([128, CT * 128], BF16)     # k heads 0,1
        nc.sync.dma_start(wk_sb[:], wkP[:])
        xq = consts.tile([128, CT, kp], BF16)          # x compact, transposed
        for ct in range(CT):
            nc.sync.dma_start(xq[:, ct, :], xcT_t[ct])
        wq_sb = consts.tile([128, CT * 128], BF16)     # q heads 0,1
        nc.sync.dma_start(wq_sb[:], wqP[:])
        wqk_sb = consts.tile([128, CT * 128], BF16)    # q head 2 | k head 2
        nc.sync.dma_start(wqk_sb[:], wqkS[:])
        wv_sb = consts.tile([128, CT * HD], BF16)
        nc.sync.dma_start(wv_sb[:], wvP[:])
        kcb_sb = consts.tile([128, kt_n], F32)         # exp bias per key
        nc.sync.dma_start(kcb_sb[:], kcb[:])
        wo_sb = consts.tile([128, 2 * C], BF16)        # rows 0..127 | 128..191
        nc.sync.dma_start(wo_sb[:], woP[:])            # solo row 64 = bo/4

        # warm the Exp activation table so the first real exp skips the
        # 1.28us table load
        dum_f = consts.tile([1, 2], F32)
        nc.vector.memset(dum_f[:], 0.0)
        dum_o = consts.tile([1, 2], BF16)
        nc.scalar.activation(dum_o[:], dum_f[:],
                             mybir.ActivationFunctionType.Exp,
                             bias=0.0, scale=1.0)


        # RS buffers carry 16 pad columns so the final rs_out->out copy has
        # a 2D row-wise descriptor pattern (288 x 1536B) instead of a few
        # giant flattened descriptors.
        CP = C + 16
        kr = -(-cnt // 4) * 4          # rows the collective actually reduces
        rs_in = dram.tile([kp, CP], RSDT)
        rs_out = dram.tile([kp // 4, CP], RSDT)

        q_pair = consts.tile([128, kp], BF16, tag="q_pair")   # heads 0,1
        k_pair = consts.tile([128, kp], BF16, tag="k_pair")
        q_solo = consts.tile([64, kp], BF16, tag="q_solo")    # head 2
        k_solo = consts.tile([64, kp], BF16, tag="k_solo")
        v_aug = consts.tile([128, kt_n, HPG, D + 1], BF16, tag="v_aug")
        nc.vector.memset(v_aug[:, :, :, D:D + 1], 1.0)        # sums column

        def proj_chunk(w_sb, s, w, copies):
            ps = p512.tile([128, 512], F32, tag="p512")
            for ct in range(CT):
                nc.tensor.matmul(ps[:, :w], w_sb[:, ct * 128:(ct + 1) * 128],
                                 xq[:, ct, s:s + w],
                                 start=(ct == 0), stop=(ct == CT - 1))
            for (eng, dst, lo, hi) in copies:
                eng(dst[:, s:s + w], ps[lo:hi, :w])

        def emit_v(kt):
            ps = p512.tile([128, 512], F32, tag="p512")
            for ct in range(CT):
                nc.tensor.matmul(ps[:, 0:HD],
                                 xq[:, ct, kt * 128:(kt + 1) * 128],
                                 wv_sb[:, ct * HD:(ct + 1) * HD],
                                 start=(ct == 0), stop=(ct == CT - 1))
            nc.vector.tensor_copy(
                v_aug[:, kt, :, 0:D],
                ps[:, 0:HD].rearrange("p (h d) -> p h d", h=HPG))

        # ---- upfront projections: all of k, q over span0 ----------------
        chunks = _qchunks(kp, kp, 512)
        for (s, w) in chunks:
            proj_chunk(wk_sb, s, w, [(nc.scalar.copy, k_pair, 0, 128)])
        for (s, w) in chunks:
            if s < span0_end:
                proj_chunk(wq_sb, s, w,
                           [(nc.vector.tensor_copy, q_pair, 0, 128)])
        # deferred: solo q/k and the q tail, emitted inside h1's exp bubbles
        # (GPSIMD cannot read PSUM on HW, so these copies go to DVE)
        # solo chunks inside span0's column range must land before h2's
        # first scores (the moving q operand spans all span0 columns), so
        # they ride h1's bubbles; tail chunks are only needed by span1 and
        # ride h2's bubbles instead of overflowing h1.
        fillers = []
        fillers2 = []
        for (s, w) in chunks:
            tgt = fillers if s < span0_end else fillers2
            tgt.append(lambda s=s, w=w: proj_chunk(
                wqk_sb, s, w, [(nc.vector.tensor_copy, q_solo, 0, 64),
                               (nc.vector.tensor_copy, k_solo, 64, 128)]))
        for (s, w) in chunks:
            if s >= span0_end:
                fillers2.append(lambda s=s, w=w: proj_chunk(
                    wq_sb, s, w, [(nc.vector.tensor_copy, q_pair, 0, 128)]))

        HSRC = [(k_pair, 0, q_pair, 0, None, 0),
                (k_pair, 64, q_pair, 64, None, 64),
                (k_solo, 0, q_solo, 0, None, 0)]

        # ---- attention + projection, span-major --------------------------
        first_span = True
        deferred = []                  # span0 pj blocks, run in span1 bubbles
        for si, (qs, qw) in enumerate(spans):
            is_last = (si == len(spans) - 1)
            on_pair = onsb.tile([128, span_w], BF16, tag="on_pair")
            on_solo = onsb.tile([65, span_w], BF16, tag="on_solo")
            nc.vector.memset(on_solo[64:65, :], 1.0)   # x1 row: bias via wo

            def on_dst_of(h):
                return (on_pair, 64 * h) if h < 2 else (on_solo, 0)

            def norm(h, o_ps, off, nch=1):
                on_dst, on_lo = on_dst_of(h)
                cwd = -(-qw // nch)
                rec = asb.tile([1, span_w], F32, tag="rec")
                rec_bc = asb.tile([D, span_w], F32, tag="rec_bc")
                for c0 in range(0, qw, cwd):
                    c1 = min(qw, c0 + cwd)
                    nc.vector.reciprocal(rec[:, c0:c1],
                                         o_ps[D:D + 1, off + c0:off + c1])
                    nc.gpsimd.partition_broadcast(rec_bc[:, c0:c1],
                                                  rec[:, c0:c1])
                    nc.vector.tensor_mul(on_dst[on_lo:on_lo + D, c0:c1],
                                         o_ps[0:D, off + c0:off + c1],
                                         rec_bc[:, c0:c1])

            if first_span or qw > 128:
                # -- wide span: per-head pipeline, v/solo emitted in bubbles
                for h in range(HPG):
                    k_src, k_lo, q_src, q_lo = HSRC[h][:4]
                    o_ps = ops.tile([D + 1, span_w], F32, tag="o")

                    def emit_scores(kt):
                        s_ps = aps.tile([128, span_w], F32, tag="s")
                        for ms in range(0, qw, 512):
                            mw = min(512, qw - ms)
                            nc.tensor.matmul(
                                s_ps[:, ms:ms + mw],
                                k_src[k_lo:k_lo + D, kt * 128:(kt + 1) * 128],
                                q_src[q_lo:q_lo + D, qs + ms:qs + ms + mw],
                                start=True, stop=True)
                        attn = asb.tile([128, span_w], BF16, tag="attn")
                        nc.scalar.activation(attn[:, :qw], s_ps[:, :qw],
                                             mybir.ActivationFunctionType.Exp,
                                             bias=kcb_sb[:, kt:kt + 1],
                                             scale=SCALE)
                        return attn

                    def emit_av(kt, attn):
                        for ms in range(0, qw, 512):
                            mw = min(512, qw - ms)
                            nc.tensor.matmul(
                                o_ps[:, ms:ms + mw],
                                v_aug[:, kt, h, :],
                                attn[:, ms:ms + mw],
                                start=(kt == 0), stop=(kt == kt_n - 1))

                    def filler(kt):
                        if first_span and h == 0:
                            emit_v(kt)          # v projection rides h0 bubbles
                        elif first_span and h == 1 and fillers:
                            fillers.pop(0)()
                        elif first_span and h == 2 and fillers2:
                            fillers2.pop(0)()
                        elif not first_span and h >= 1 and deferred:
                            deferred.pop(0)()

                    # software pipeline: scores(kt+1) before av(kt), so PE
                    # never waits on exp(kt) in its in-order queue
                    attn_prev = emit_scores(0)
                    filler(0)
                    for kt in range(1, kt_n):
                        attn_cur = emit_scores(kt)
                        filler(kt)
                        emit_av(kt - 1, attn_prev)
                        attn_prev = attn_cur
                    emit_av(kt_n - 1, attn_prev)
                    if first_span and h == 1:
                        while fillers:      # flush leftover span0 solo chunks
                            fillers.pop(0)()
                    if first_span and h == HPG - 1:
                        while fillers2:     # flush leftover tail chunks
                            fillers2.pop(0)()
                    norm(h, o_ps, 0,
                         nch=8 if (h == HPG - 1 and qw > 256) else 1)
                if not first_span:
                    while deferred:
                        deferred.pop(0)()
            else:
                # -- narrow last span: all 27 scores+exp first, with span0's
                # deferred pj blocks riding the exp bubbles. Two scores pack
                # into each s tile at bank-aligned offsets (0 / 2048B), so 4
                # are in flight instead of 2 and the exp latency chain
                # halves. avs then accumulate into three INDEPENDENT psum
                # tiles (heads 0/1 borrow the now-idle s ring), so the three
                # norm chains overlap instead of serializing on the o ring.
                sc_attn = {}
                units = [(h, kt) for h in range(HPG) for kt in range(kt_n)]
                cur = None
                for u, (h, kt) in enumerate(units):
                    if u % 2 == 0 or cur is None:
                        cur = aps.tile([128, span_w], F32, tag="s")
                    band = (u % 2) * 512 if span_w >= 512 + qw else 0
                    k_src, k_lo, q_src, q_lo = HSRC[h][:4]
                    nc.tensor.matmul(
                        cur[:, band:band + qw],
                        k_src[k_lo:k_lo + D, kt * 128:(kt + 1) * 128],
                        q_src[q_lo:q_lo + D, qs:qs + qw],
                        start=True, stop=True)
                    attn = nsb.tile([128, 128], BF16, tag="attn1")
                    nc.scalar.activation(attn[:, :qw],
                                         cur[:, band:band + qw],
                                         mybir.ActivationFunctionType.Exp,
                                         bias=kcb_sb[:, kt:kt + 1],
                                         scale=SCALE)
                    sc_attn[h, kt] = attn
                    if deferred:
                        deferred.pop(0)()
                while deferred:
                    deferred.pop(0)()
                o_tiles = []
                for h in range(HPG):
                    if h < 2:
                        o_ps = aps.tile([128, span_w], F32, tag="s")
                    else:
                        o_ps = ops.tile([D + 1, span_w], F32, tag="o")
                    for kt in range(kt_n):
                        nc.tensor.matmul(o_ps[0:D + 1, 0:qw],
                                         v_aug[:, kt, h, :],
                                         sc_attn[h, kt][:, 0:qw],
                                         start=(kt == 0),
                                         stop=(kt == kt_n - 1))
                    o_tiles.append(o_ps)
                for h in range(HPG):
                    norm(h, o_tiles[h], 0)

            # -- output projection; bias is row 64 of the solo wo slice,
            # hit by on_solo's constant ones row, so the PSUM->SBUF move is
            # a plain copy. For non-final spans the blocks are deferred into
            # the next span's exp bubbles (DVE movers, off the Act chain).
            def emit_pj_block(qc, cw, cs, ccw, mv, qs=qs, on_pair=on_pair,
                              on_solo=on_solo):
                pj = p512.tile([128, 512], F32, tag="p512")
                nc.tensor.matmul(pj[:cw, :ccw],
                                 on_pair[:, qc:qc + cw],
                                 wo_sb[:, cs:cs + ccw],
                                 start=True, stop=False)
                nc.tensor.matmul(pj[:cw, :ccw],
                                 on_solo[0:65, qc:qc + cw],
                                 wo_sb[0:65, C + cs:C + cs + ccw],
                                 start=False, stop=True)
                pj_sb = jsb.tile([128, 512], RSDT, tag="pj_sb")
                mv(pj_sb[:cw, :ccw], pj[:cw, :ccw])
                nc.sync.dma_start(
                    rs_in[qs + qc:qs + qc + cw, cs:cs + ccw],
                    pj_sb[:cw, :ccw])

            movers = [nc.vector.tensor_copy, nc.scalar.copy]
            bi = 0
            for qi, qc in enumerate(range(0, qw, 128)):
                cw = min(128, qw - qc)
                for ci, cs in enumerate(range(0, C, 512)):
                    ccw = min(512, C - cs)
                    mvd = movers[bi % 2]   # alternate so each engine gets a
                    bi += 1                # mix of wide and narrow copies
                    if is_last:
                        emit_pj_block(qc, cw, cs, ccw, mvd)
                    else:
                        deferred.append(
                            lambda qc=qc, cw=cw, cs=cs, ccw=ccw, mvd=mvd,
                                   f=emit_pj_block:
                            f(qc, cw, cs, ccw, mvd))
            first_span = False

        # ---- reduce-scatter over the 4 cores of this batch ---------------
        # bf16 payload, scattered straight into the ExternalOutput: no
        # post-collective DRAM->DRAM copy.
        if with_rs:
            nc.gpsimd.collective_compute(
                "ReduceScatter", mybir.AluOpType.add,
                replica_groups=[[0, 1, 2, 3], [4, 5, 6, 7]],
                ins=[rs_in[0:kr, :]], outs=[rs_out[0:kr // 4, :]])
            nc.sync.dma_start(out_ext[0:kr // 4, :], rs_out[0:kr // 4, 0:C])
        else:
            nc.sync.dma_start(out_ext[:], rs_in[0:kp // 4, :])


def _pack_w(a):
    """[n_out, C] weight slice -> SBUF stationary layout [128, CT*n_out]:
    partition p, block ct, col d  =  a[d, ct*128 + p]."""
    n = a.shape[0]
    return np.ascontiguousarray(
        a.reshape(n, CT, 128).transpose(2, 1, 0).reshape(128, CT * n))


def make_in_maps(inputs, kept, cnt, kp):
    x = np.asarray(inputs["x"], dtype=np.float32)
    Wq, Wk, Wv, Wo = (np.asarray(inputs[k], np.float32)
                      for k in ("Wq", "Wk", "Wv", "Wo"))
    bo = np.asarray(inputs["bo"], np.float32)
    woT_full = np.ascontiguousarray(Wo.T)          # [hd_in, c_out]
    in_maps = []
    for c in range(NCORES):
        b, g = divmod(c, NGROUPS)
        hs = slice(g * HD, (g + 1) * HD)
        xc = np.zeros((kp, C), np.float32)
        xc[:cnt[b]] = x[b][kept[b]]
        kcb_flat = np.full(kp, -1000.0, np.float32)
        kcb_flat[:cnt[b]] = 0.0
        kcb = np.ascontiguousarray(kcb_flat.reshape(kp // 128, 128).T)
        wq_s, wk_s, wv_s = Wq[hs], Wk[hs], Wv[hs]
        wo_s = woT_full[hs]                        # [192, C]
        woP = np.zeros((128, 2 * C), np.float32)
        woP[:, :C] = wo_s[0:128]
        woP[0:64, C:] = wo_s[128:HD]
        woP[64, C:] = bo / NGROUPS     # bias row, hit by on_solo's ones row
        in_maps.append({
            "xcT": np.ascontiguousarray(xc.T).astype(NPBF16),
            "kcb": kcb,
            "wqP": _pack_w(wq_s[0:128]).astype(NPBF16),
            "wkP": _pack_w(wk_s[0:128]).astype(NPBF16),
            "wqkS": _pack_w(
                np.concatenate([wq_s[128:HD], wk_s[128:HD]], 0)).astype(NPBF16),
            "wvP": _pack_w(wv_s).astype(NPBF16),
            "woP": woP.astype(NPBF16),
        })
    return in_maps


def sim_exec_ns(**inputs):
    """CoreSim cost-model end-to-end time (ns) for the compiled program.
    Test-harness helper only; not used by kernel()."""
    from concourse.bass_interp import MultiCoreSim
    x = np.asarray(inputs["x"], np.float32)
    mask = np.asarray(inputs["mask"])
    kept = [np.nonzero(mask[b])[0] for b in range(B)]
    cnt = [len(k) for k in kept]
    cnt_max = max(max(cnt), 1)
    kp = max(128, -(-cnt_max // 128) * 128)
    nc = _build(kp, cnt_max)
    in_maps = make_in_maps(inputs, kept, cnt, kp)
    sim = MultiCoreSim(nc, num_cores=NCORES, num_workers=1,
                       require_finite=False, require_nnan=False)
    for core_id, m in enumerate(in_maps):
        for name, val in m.items():
            sim.cores[core_id].tensor(name)[:] = val
    sim.simulate()
    return max(core.time for core in sim.cores.values())


def mvproj_rows(x, Wq_unused, Wv, Wo, bo):
    """fp32 host path for fully-masked queries: uniform attention over all
    keys reduces to Wo @ (Wv @ mean(x)) + bo, identical for every such row."""
    return np.stack([Wo @ (Wv @ x[b].mean(0)) + bo for b in range(B)])


def kernel(x, mask, Wq, Wk, Wv, Wo, bo):
    x = np.asarray(x, dtype=np.float32)
    mask = np.asarray(mask)
    kept = [np.nonzero(mask[b])[0] for b in range(B)]
    cnt = [len(k) for k in kept]
    cnt_max = max(max(cnt), 1)
    kp = max(128, -(-cnt_max // 128) * 128)

    nc = _build(kp, cnt_max)
    in_maps = make_in_maps(
        {"x": x, "Wq": Wq, "Wk": Wk, "Wv": Wv, "Wo": Wo, "bo": bo},
        kept, cnt, kp)

    r = run_bass_kernel_spmd(nc, in_maps, core_ids=list(range(NCORES)))
    global LAST_HW_NS, LAST_RESULT
    LAST_RESULT = r
    if getattr(r, "exec_time_ns", None):
        LAST_HW_NS = r.exec_time_ns

    Wv32 = np.asarray(Wv, np.float32)
    Wo32 = np.asarray(Wo, np.float32)
    bo32 = np.asarray(bo, np.float32)
    mvp = mvproj_rows(x, None, Wv32, Wo32, bo32)

    kr4 = (-(-cnt_max // 4) * 4) // 4  # valid rows per core (RS row trim)
    out = np.empty((B, N, C), np.float32)
    for b in range(B):
        rs = np.concatenate([np.asarray(r.results[NGROUPS * b + i]["out"],
                                        np.float32)[:kr4]
                             for i in range(NGROUPS)], axis=0)
        out[b, kept[b]] = rs[:cnt[b]]
        out[b, mask[b] == 0] = mvp[b]
    return out

